# revision 23
# baseline (speedup 1.0000x reference)
"""Trainium2 Bass kernel for nn_DiTBlock (HGRN-attention DiT block).

Sharding: 8 cores = 4 batches x 2 half-sequences (1024 tokens each).
All bitlinear matmuls are exact integer arithmetic: activations quantized
to int8-range integers stored in bf16, ternary weights stored in fp8e4
(exact, half the DMA of bf16). The time recurrence h_t = f_t*h_{t-1} + i_t
runs on the DVE tensor_tensor_scan (512-wide chunks chained via AP
initial); the half-sequence boundary carry crosses cores via one AllGather
whose latency is hidden under the independent g-projection matmuls. adaln
params are computed locally per-core with a 3-pass split-bf16 matmul
(fp32-accurate weights+activations), no collective.

Memory: one 4-slot rotating pool holds every 32KB-class intermediate
(x_res, moda, xqT, ha, cam, gs, hT, oa, oqT, x2qT, xn_all, mod2, h2,
h2qT) — slot reuse follows phase order so the WAR deps the rotation
inserts are all already satisfied. PSUM rotates matmul groups over 4
banks so accumulation groups pipeline.
"""
import functools
import numpy as np
import ml_dtypes

import concourse.bass as bass
import concourse.bacc as bacc_mod
import concourse.mybir as mybir
import concourse.tile as tile
from concourse.masks import make_identity
from concourse.bass_utils import run_bass_kernel_spmd

BF16 = ml_dtypes.bfloat16
FP8 = ml_dtypes.float8_e4m3fn
F32 = mybir.dt.float32
BF = mybir.dt.bfloat16
F8 = mybir.dt.float8e4
U32 = mybir.dt.uint32
AL = mybir.AluOpType
AF = mybir.ActivationFunctionType
AX = mybir.AxisListType

B, T, D = 4, 2048, 1024
TOK = 1024          # tokens per core
NH, HD = 16, 64
MLP = 4096
N_CORES = 8
C_MAGIC = float(1.5 * 2 ** 23)
MAGIC_U32 = 0x5F3759DF


def _quant_w(w):
    invws = float(np.clip(np.abs(w).mean(dtype=np.float64), 1e-5, None))
    m = np.clip(np.round(w.astype(np.float64) / invws), -1, 1).astype(np.float32)
    return m, np.float32(invws)


def _rsqrt(nc, sb, x_ap, scale, bias, shape, tag):
    """out = rsqrt(x*scale + bias), Newton on DVE. Returns a new tile."""
    t = sb.tile(shape, F32, tag=tag + "_t", name=tag + "_t")
    nc.vector.tensor_scalar(out=t, in0=x_ap, scalar1=float(scale),
                            scalar2=float(bias), op0=AL.mult, op1=AL.add)
    y = sb.tile(shape, F32, tag=tag + "_y", name=tag + "_y")
    sh = sb.tile(shape, F32, tag=tag + "_s", name=tag + "_s")
    nc.vector.tensor_scalar(out=sh[:].bitcast(U32), in0=t[:].bitcast(U32),
                            scalar1=1, scalar2=None, op0=AL.logical_shift_right)
    mg = sb.tile(shape, F32, tag=tag + "_m", name=tag + "_m")
    nc.vector.memset(mg[:].bitcast(U32), MAGIC_U32)
    nc.vector.tensor_tensor(out=y[:].bitcast(U32), in0=mg[:].bitcast(U32),
                            in1=sh[:].bitcast(U32), op=AL.subtract)
    e = sb.tile(shape, F32, tag=tag + "_e", name=tag + "_e")
    for _ in range(3):
        nc.vector.tensor_tensor(out=e, in0=y, in1=y, op=AL.mult)
        nc.vector.tensor_tensor(out=e, in0=e, in1=t, op=AL.mult)
        nc.vector.tensor_scalar(out=e, in0=e, scalar1=-0.5, scalar2=1.5,
                                op0=AL.mult, op1=AL.add)
        nc.vector.tensor_tensor(out=y, in0=y, in1=e, op=AL.mult)
    return y


def _build(iw):
    """iw: dict of invws floats. Returns finalized Bacc program."""
    nc = bacc_mod.Bacc("TRN2", target_bir_lowering=False)

    x_sl = nc.declare_dram_parameter("x_sl", [TOK, D], F32, isOutput=False)
    c_col = nc.declare_dram_parameter("c_col", [128, 8], F32, isOutput=False)
    adw_hi = nc.declare_dram_parameter("adw_hi", [12, 128, 8, 512], BF,
                                       isOutput=False)
    adw_lo = nc.declare_dram_parameter("adw_lo", [12, 128, 8, 512], BF,
                                       isOutput=False)
    adb_row = nc.declare_dram_parameter("adb_row", [12, 1, 512], F32,
                                        isOutput=False)
    mask8 = nc.declare_dram_parameter("mask8", [N_CORES, 1], F32, isOutput=False)
    gnr = nc.declare_dram_parameter("gnr", [1, D], F32, isOutput=False)
    wi4 = nc.declare_dram_parameter("wi4", [8, 128, 8, 128], F8, isOutput=False)
    wf4 = nc.declare_dram_parameter("wf4", [8, 128, 8, 128], F8, isOutput=False)
    wg3 = nc.declare_dram_parameter("wg3", [128, 8, D], F8, isOutput=False)
    wo3 = nc.declare_dram_parameter("wo3", [128, 8, D], F8, isOutput=False)
    gw4 = nc.declare_dram_parameter("gw4", [8, 128, 8, 1024], F8, isOutput=False)
    dw3 = nc.declare_dram_parameter("dw3", [128, 32, D], F8, isOutput=False)
    out_sl = nc.declare_dram_parameter("out_sl", [TOK, D], F32, isOutput=True)

    cc2_in = nc.dram_tensor("cc2_in", [D], F32)
    cc2_out = nc.dram_tensor("cc2_out", [N_CORES, D], F32, addr_space="Shared")
    dqrow_d = nc.dram_tensor("dqrow_d", [D], F32)
    xnew_d = nc.dram_tensor("xnew_d", [TOK, D], F32)

    RG = [list(range(N_CORES))]

    with tile.TileContext(nc) as tc:
        import contextlib
        es = contextlib.ExitStack()
        with es:
            # ---------- pools ----------
            cst = es.enter_context(tc.tile_pool(name="cst", bufs=1))
            ps = es.enter_context(tc.tile_pool(name="ps", bufs=1, space="PSUM"))
            big_pool = es.enter_context(tc.tile_pool(name="big", bufs=1))

            def pmm(tag="mm", shape=(128, 512)):
                return ps.tile(list(shape), F32, tag=tag, name=tag, bufs=4)

            def ptp():
                return ps.tile([128, 512], BF, tag="tp", name="tp", bufs=2)

            def ptpf():
                return ps.tile([128, 512], F32, tag="tpf", name="tpf", bufs=2)

            def big(shape, dtype, name):
                # 4 rotating 32KB slots; creation order == phase order
                return big_pool.tile(shape, dtype, tag="bigslot", name=name,
                                     bufs=4)

            # constants (small)
            identb = cst.tile([128, 128], BF)
            make_identity(nc, identb)
            identf = cst.tile([128, 128], F32)
            make_identity(nc, identf)
            ones_row = cst.tile([1, 128], F32)
            nc.vector.memset(ones_row, 1.0)
            mask_sb = cst.tile([N_CORES, 1], F32)
            nc.sync.dma_start(out=mask_sb, in_=mask8[:, :])
            negC = cst.tile([128, 1], F32)
            nc.vector.memset(negC, -C_MAGIC)
            q127A = cst.tile([128, 8], F32); dqA = cst.tile([128, 8], F32)
            dqAg = cst.tile([128, 8], F32)
            q127O = cst.tile([128, 8], F32); dqOo = cst.tile([128, 8], F32)
            q127C = cst.tile([128, 8], F32); dqCg = cst.tile([128, 8], F32)
            B_g2 = cst.tile([128, D], F32)

            # big-slot tiles (created lazily in phase order; slot = index % 4):
            #  s0: x_res -> cam -> oqT -> h2a
            #  s1: moda  -> gs  -> x2qT -> (dummy)
            #  s2: xqT   -> hT  -> xn_all -> h2b
            #  s3: ha    -> oa  -> mod2 -> h2qT
            x_res = big([128, 8, D], F32, "x_res")      # s0: phase0..A

            # outer small pools (LIFO): pLate > pGn > pSb > pShSc
            pLate = tc.tile_pool(name="pLate", bufs=1)
            late = pLate.__enter__()                    # closes after C
            B_g1 = late.tile([128, D], F32)
            B_sh2 = late.tile([128, D], F32)
            B_sc2 = late.tile([128, D], F32)
            pGn = tc.tile_pool(name="pGn", bufs=1)
            gn_pool = pGn.__enter__()                   # closes after g-stage
            B_gn = gn_pool.tile([128, D], F32)
            pSb = tc.tile_pool(name="pSb", bufs=1)
            sb_pool = pSb.__enter__()                   # closes after B
            Sb_i = sb_pool.tile([128, D], F32)
            Sb_f = sb_pool.tile([128, D], F32)
            pShSc = tc.tile_pool(name="pShSc", bufs=1)
            shsc = pShSc.__enter__()                    # closes after A
            B_sh1 = shsc.tile([128, D], F32)
            B_sc1 = shsc.tile([128, D], F32)

            # resident x (single strided DMA)
            nc.sync.dma_start(out=x_res,
                              in_=x_sl[:, :].rearrange("(i p) c -> p i c", p=128))

            # ---------------- phase 0: adaln (local) + LN1 stats ----------
            with tc.tile_pool(name="p0", bufs=2) as wk:
                gnr_sb = wk.tile([1, D], F32, tag="gnrs", bufs=1)
                nc.sync.dma_start(out=gnr_sb, in_=gnr[:, :])
                c_sb = wk.tile([128, 8], F32, tag="csb", bufs=1)
                nc.sync.dma_start(out=c_sb, in_=c_col[:, :])
                cs_sb = wk.tile([128, 8], F32, tag="cssb", bufs=1)
                nc.scalar.activation(out=cs_sb, in_=c_sb, func=AF.Silu)
                cs_hi = wk.tile([128, 8], BF, tag="cshi", bufs=1)
                nc.vector.tensor_copy(out=cs_hi, in_=cs_sb)
                cs_hif = wk.tile([128, 8], F32, tag="cshif", bufs=1)
                nc.vector.tensor_copy(out=cs_hif, in_=cs_hi)
                cs_lo = wk.tile([128, 8], BF, tag="cslo", bufs=1)
                nc.vector.tensor_tensor(out=cs_lo, in0=cs_sb, in1=cs_hif,
                                        op=AL.subtract)

                # chunk ch -> destination broadcast tile slice
                bdst = {0: (B_sh1, 0, False), 1: (B_sh1, 512, False),
                        2: (B_sc1, 0, True), 3: (B_sc1, 512, True),
                        4: (B_g1, 0, False), 5: (B_g1, 512, False),
                        6: (B_sh2, 0, False), 7: (B_sh2, 512, False),
                        8: (B_sc2, 0, True), 9: (B_sc2, 512, True),
                        10: (B_g2, 0, False), 11: (B_g2, 512, False)}

                def adaln_chunks(ch_list, three_pass):
                    # params = cs_hi@Whi [+ cs_hi@Wlo + cs_lo@Whi]  (+bias)
                    for ch in ch_list:
                        adwh_c = wk.tile([128, 8, 512], BF, tag="adwh")
                        nc.sync.dma_start(out=adwh_c, in_=adw_hi[ch])
                        if three_pass:
                            adwl_c = wk.tile([128, 8, 512], BF, tag="adwl",
                                             bufs=1)
                            nc.sync.dma_start(out=adwl_c, in_=adw_lo[ch])
                        adb_c = wk.tile([1, 512], F32, tag="adbc")
                        nc.sync.dma_start(out=adb_c, in_=adb_row[ch])
                        pa_ps = pmm("mm", (1, 512))
                        for j in range(8):
                            nc.tensor.matmul(pa_ps, cs_hi[:, j:j + 1],
                                             adwh_c[:, j, :],
                                             start=(j == 0),
                                             stop=(j == 7 and not three_pass))
                        if three_pass:
                            for j in range(8):
                                nc.tensor.matmul(pa_ps, cs_hi[:, j:j + 1],
                                                 adwl_c[:, j, :],
                                                 start=False, stop=False)
                            for j in range(8):
                                nc.tensor.matmul(pa_ps, cs_lo[:, j:j + 1],
                                                 adwh_c[:, j, :],
                                                 start=False, stop=(j == 7))
                        prow = wk.tile([1, 512], F32, tag="prow")
                        nc.vector.tensor_tensor(out=prow, in0=pa_ps, in1=adb_c,
                                                op=AL.add)
                        dst, off, plus1 = bdst[ch]
                        pb_ps = pmm("mm")
                        nc.tensor.matmul(pb_ps, ones_row, prow,
                                         start=True, stop=True)
                        if plus1:
                            nc.scalar.activation(out=dst[:, off:off + 512],
                                                 in_=pb_ps, func=AF.Identity,
                                                 bias=1.0)
                        else:
                            nc.scalar.copy(out=dst[:, off:off + 512], in_=pb_ps)

                # shift_msa/scale_msa first so modulate can start early
                adaln_chunks(range(0, 4), True)

                # LN1 stats (pure DVE, overlaps adaln matmuls)
                muA = shsc.tile([128, 8], F32)
                varA = shsc.tile([128, 8], F32)
                for i in range(8):
                    st = wk.tile([128, 2, 6], F32, tag="bst")
                    xr = x_res[:, i, :].rearrange("p (s d) -> p s d", s=2)
                    for s2 in range(2):
                        nc.vector.bn_stats(out=st[:, s2, :], in_=xr[:, s2, :])
                    mv = wk.tile([128, 2], F32, tag="bmv")
                    nc.vector.bn_aggr(out=mv, in_=st)
                    nc.vector.tensor_copy(out=muA[:, i:i + 1], in_=mv[:, 0:1])
                    nc.vector.tensor_copy(out=varA[:, i:i + 1], in_=mv[:, 1:2])
                rstdLN = _rsqrt(nc, shsc, varA, 1.0, 1e-6, [128, 8], "rLN")
                nmr = shsc.tile([128, 8], F32)
                nc.vector.tensor_tensor(out=nmr, in0=muA, in1=rstdLN, op=AL.mult)
                nc.vector.tensor_scalar(out=nmr, in0=nmr, scalar1=-1.0,
                                        scalar2=None, op0=AL.mult)

                adaln_chunks(range(4, 12), True)

                # gnorm broadcast
                for ch in range(0, D, 512):
                    pb_ps = pmm("mm")
                    nc.tensor.matmul(pb_ps, ones_row, gnr_sb[:, ch:ch + 512],
                                     start=True, stop=True)
                    nc.scalar.copy(out=B_gn[:, ch:ch + 512], in_=pb_ps)

            # ---------------- helpers ----------------
            def quant_batch(amx, ssx, n, dk, q127, dqt, dq_scaled, iws_scaled,
                            sb_p, tagp):
                """q127 = 127/max(amx,1e-5); dqt = amc*rsqrt(ssx/dk+1e-8)/127."""
                shape = [128, n]
                amc = sb_p.tile(shape, F32, tag=tagp + "amc", name=tagp + "amc")
                nc.vector.tensor_scalar(out=amc, in0=amx, scalar1=1e-5,
                                        scalar2=None, op0=AL.max)
                rec = sb_p.tile(shape, F32, tag=tagp + "rec", name=tagp + "rec")
                nc.vector.reciprocal(out=rec, in_=amc)
                nc.vector.tensor_scalar(out=q127, in0=rec, scalar1=127.0,
                                        scalar2=None, op0=AL.mult)
                rs = _rsqrt(nc, sb_p, ssx, 1.0 / dk, 1e-8, shape, tagp + "rs")
                nc.vector.tensor_tensor(out=dqt, in0=amc, in1=rs, op=AL.mult)
                nc.vector.tensor_scalar(out=dqt, in0=dqt, scalar1=1.0 / 127.0,
                                        scalar2=None, op0=AL.mult)
                if dq_scaled is not None:
                    nc.vector.tensor_scalar(out=dq_scaled, in0=dqt,
                                            scalar1=float(iws_scaled),
                                            scalar2=None, op0=AL.mult)

            def round_and_transpose(src, q_col, dst_bf, i, nblk, sb_p, tagp):
                """round src [128, 128*nblk] -> bf16, transpose 128-blocks into
                dst_bf[:, j, 128i:...]. Processes in <=1024-wide sub-chunks."""
                for c0 in range(0, nblk, 8):
                    nb8 = min(8, nblk - c0)
                    w = 128 * nb8
                    t2 = sb_p.tile([128, 1024], F32, bufs=1, tag=tagp + "t2",
                                   name=tagp + "t2")
                    nc.vector.tensor_scalar(out=t2[:, 0:w],
                                            in0=src[:, 128 * c0:128 * c0 + w],
                                            scalar1=q_col, scalar2=C_MAGIC,
                                            op0=AL.mult, op1=AL.add)
                    kq = sb_p.tile([128, 1024], BF, bufs=2, tag=tagp + "kq",
                                   name=tagp + "kq")
                    nc.scalar.activation(out=kq[:, 0:w], in_=t2[:, 0:w],
                                         func=AF.Identity, bias=negC)
                    for g4 in range(0, nb8, 4):
                        nb = min(4, nb8 - g4)
                        tp = ptp()
                        for jj in range(nb):
                            nc.tensor.transpose(
                                tp[:, 128 * jj:128 * (jj + 1)],
                                kq[:, 128 * (g4 + jj):128 * (g4 + jj + 1)],
                                identb)
                        for jj in range(nb):
                            dst = dst_bf[:, c0 + g4 + jj, 128 * i:128 * (i + 1)]
                            if jj % 2 == 0:
                                nc.scalar.copy(out=dst,
                                               in_=tp[:, 128 * jj:128 * (jj + 1)])
                            else:
                                nc.vector.tensor_copy(
                                    out=dst, in_=tp[:, 128 * jj:128 * (jj + 1)])

            # ---------------- phase A: modulate + quant ----------------
            moda = big([128, 8, D], F32, "moda")        # s1: A
            xqT = big([128, 8, D], BF, "xqT")           # s2: A..g
            with tc.tile_pool(name="pa", bufs=2) as pa:
                amA = shsc.tile([128, 8], F32)
                ssA = shsc.tile([128, 8], F32)
                for i in range(8):
                    u = pa.tile([128, D], F32, tag="u", bufs=1)
                    nc.scalar.activation(out=u, in_=x_res[:, i, :],
                                         func=AF.Identity,
                                         scale=rstdLN[:, i:i + 1],
                                         bias=nmr[:, i:i + 1])
                    tt = pa.tile([128, D], F32, tag="tt", bufs=1)
                    nc.vector.tensor_tensor(out=tt, in0=u, in1=B_sc1, op=AL.mult)
                    nc.vector.tensor_tensor(out=moda[:, i, :], in0=tt, in1=B_sh1,
                                            op=AL.add)
                    nc.vector.tensor_reduce(out=amA[:, i:i + 1], in_=moda[:, i, :],
                                            axis=AX.X, op=AL.max,
                                            apply_absolute_value=True)
                    scr = pa.tile([128, D], F32, tag="sq", bufs=1)
                    nc.scalar.activation(out=scr, in_=moda[:, i, :], func=AF.Square,
                                         accum_out=ssA[:, i:i + 1])
                quant_batch(amA, ssA, 8, D, q127A, dqA, dqAg, iw["g"], pa, "qa")
                # dq row via DRAM bounce, then Sb_i / Sb_f broadcasts
                nc.sync.dma_start(out=dqrow_d[:].rearrange("(i p) -> p i", p=128),
                                  in_=dqA)
                dqrow_sb = pa.tile([1, D], F32, tag="dqrow", bufs=1)
                nc.sync.dma_start(out=dqrow_sb,
                                  in_=dqrow_d[:].rearrange("(one d) -> one d",
                                                           one=1))
                oi = pa.tile([1, 128], F32, tag="oi", bufs=1)
                nc.vector.memset(oi, float(iw["i"]))
                of = pa.tile([1, 128], F32, tag="of", bufs=1)
                nc.vector.memset(of, float(iw["f"]))
                for ch in range(0, D, 512):
                    pb_ps = pmm("mm")
                    nc.tensor.matmul(pb_ps, oi, dqrow_sb[:, ch:ch + 512],
                                     start=True, stop=True)
                    nc.scalar.copy(out=Sb_i[:, ch:ch + 512], in_=pb_ps)
                    pb2 = pmm("mm")
                    nc.tensor.matmul(pb2, of, dqrow_sb[:, ch:ch + 512],
                                     start=True, stop=True)
                    nc.vector.tensor_copy(out=Sb_f[:, ch:ch + 512], in_=pb2)
                for i in range(8):
                    round_and_transpose(moda[:, i, :], q127A[:, i:i + 1], xqT,
                                        i, 8, pa, "ra")
            pShSc.__exit__(None, None, None)

            # ---------------- phase B: i/f matmuls + scan (512 chunks) -----
            ha = big([128, 8, D], F32, "ha")            # s3: B..fixup
            cam = big([128, 8, D], F32, "cam")          # s0: B..fixup
            with tc.tile_pool(name="pb", bufs=2) as pb:
                for m in range(8):
                    wf_m = pb.tile([128, 8, 128], F8, tag="wfm")
                    nc.sync.dma_start(out=wf_m, in_=wf4[m])
                    wi_m = pb.tile([128, 8, 128], F8, tag="wim")
                    nc.sync.dma_start(out=wi_m, in_=wi4[m])
                    for ck in range(0, TOK, 512):
                        pf = pmm()
                        for j in range(8):
                            nc.tensor.matmul(pf, wf_m[:, j, :],
                                             xqT[:, j, ck:ck + 512],
                                             start=(j == 0), stop=(j == 7))
                        pi = pmm()
                        for j in range(8):
                            nc.tensor.matmul(pi, wi_m[:, j, :],
                                             xqT[:, j, ck:ck + 512],
                                             start=(j == 0), stop=(j == 7))
                        ft = pb.tile([128, 512], F32, tag="ftm", bufs=2)
                        it = pb.tile([128, 512], F32, tag="itm", bufs=2)
                        nc.vector.tensor_tensor(out=ft, in0=pf,
                                                in1=Sb_f[:, ck:ck + 512],
                                                op=AL.mult)
                        nc.vector.tensor_tensor(out=it, in0=pi,
                                                in1=Sb_i[:, ck:ck + 512],
                                                op=AL.mult)
                        sigf = pb.tile([128, 512], F32, tag="sigf", bufs=2)
                        nc.scalar.activation(out=sigf, in_=ft, func=AF.Sigmoid)
                        omf = pb.tile([128, 512], F32, tag="omf", bufs=2)
                        nc.scalar.activation(out=omf, in_=ft, func=AF.Sigmoid,
                                             scale=-1.0)
                        sili = pb.tile([128, 512], F32, tag="sili", bufs=2)
                        nc.scalar.activation(out=sili, in_=it, func=AF.Silu)
                        ifin = pb.tile([128, 512], F32, tag="ifin", bufs=2)
                        nc.vector.tensor_tensor(out=ifin, in0=sili, in1=omf,
                                                op=AL.mult)
                        ha_init = 0.0 if ck == 0 else ha[:, m, ck - 1:ck]
                        nc.vector.tensor_tensor_scan(ha[:, m, ck:ck + 512],
                                                     sigf, ifin, ha_init,
                                                     op0=AL.mult, op1=AL.add)
                        cam_init = 1.0 if ck == 0 else cam[:, m, ck - 1:ck]
                        nc.vector.tensor_tensor_scan(cam[:, m, ck:ck + 512],
                                                     sigf, sigf, cam_init,
                                                     op0=AL.mult, op1=AL.bypass)
                    nc.sync.dma_start(
                        out=cc2_in[128 * m:128 * (m + 1)].rearrange(
                            "(p one) -> p one", one=1),
                        in_=ha[:, m, TOK - 1:TOK])
                nc.gpsimd.collective_compute(
                    "AllGather", AL.bypass, ins=[cc2_in[:]], outs=[cc2_out[:]],
                    replica_groups=RG)
            pSb.__exit__(None, None, None)

            # ---------------- g-stage (fills the collective) -------------
            gs = big([128, 8, D], F32, "gs")            # s1: g..o
            with tc.tile_pool(name="pg", bufs=2) as pg_pool:
                wg_sb = pg_pool.tile([128, 8, D], F8, tag="wgsb", bufs=1)
                nc.sync.dma_start(out=wg_sb, in_=wg3[:, :, :])
                for t in range(8):
                    for ck in range(0, D, 512):
                        pg = pmm()
                        for j in range(8):
                            nc.tensor.matmul(pg, xqT[:, j, 128 * t:128 * (t + 1)],
                                             wg_sb[:, j, ck:ck + 512],
                                             start=(j == 0), stop=(j == 7))
                        scr = pg_pool.tile([128, 512], F32, tag="gscr", bufs=2)
                        nc.scalar.activation(out=scr, in_=pg, func=AF.Silu,
                                             scale=dqAg[:, t:t + 1])
                        nc.vector.tensor_tensor(out=gs[:, t, ck:ck + 512],
                                                in0=scr,
                                                in1=B_gn[:, ck:ck + 512],
                                                op=AL.mult)
            pGn.__exit__(None, None, None)

            # ---------------- fixup: apply carry, transpose h -------------
            hT = big([128, 8, D], F32, "hT")            # s2: fixup..o
            with tc.tile_pool(name="pf", bufs=2) as pf_pool:
                ag2 = pf_pool.tile([N_CORES, D], F32, tag="ag2", bufs=1)
                nc.sync.dma_start(out=ag2, in_=cc2_out[:, :])
                for m in range(8):
                    pc = pmm("mm", (128, 1))
                    nc.tensor.matmul(pc, ag2[:, 128 * m:128 * (m + 1)], mask_sb,
                                     start=True, stop=True)
                    carry = pf_pool.tile([128, 1], F32, tag="carry")
                    nc.scalar.copy(out=carry, in_=pc)
                    hfix = pf_pool.tile([128, TOK], F32, tag="hfix", bufs=2)
                    nc.vector.scalar_tensor_tensor(out=hfix, in0=cam[:, m, :],
                                                   scalar=carry, in1=ha[:, m, :],
                                                   op0=AL.mult, op1=AL.add)
                    for g4 in range(0, 8, 4):
                        tp = ptpf()
                        for jj in range(4):
                            t_i = g4 + jj
                            nc.tensor.transpose(
                                tp[:, 128 * jj:128 * (jj + 1)],
                                hfix[:, 128 * t_i:128 * (t_i + 1)], identf)
                        for jj in range(4):
                            t_i = g4 + jj
                            if jj % 2 == 0:
                                nc.scalar.copy(
                                    out=hT[:, t_i, 128 * m:128 * (m + 1)],
                                    in_=tp[:, 128 * jj:128 * (jj + 1)])
                            else:
                                nc.vector.tensor_copy(
                                    out=hT[:, t_i, 128 * m:128 * (m + 1)],
                                    in_=tp[:, 128 * jj:128 * (jj + 1)])

            # ---------------- o-stage -------------------------------------
            oa = big([128, 8, D], F32, "oa")            # s3: o
            oqT = big([128, 8, D], BF, "oqT")           # s0: o..C
            with tc.tile_pool(name="po", bufs=2) as po:
                mshA = po.tile([128, 8, 16], F32, tag="msh", bufs=1)
                for t in range(8):
                    sq = po.tile([128, D], F32, tag="sqo", bufs=1)
                    nc.scalar.activation(out=sq, in_=hT[:, t, :], func=AF.Square)
                    nc.vector.tensor_reduce(
                        out=mshA[:, t, :],
                        in_=sq.rearrange("p (h d) -> p h d", h=NH),
                        axis=AX.X, op=AL.add)
                rstdH = _rsqrt(nc, po, mshA[:, :, :].rearrange("p a b -> p (a b)"),
                               1.0 / HD, 1e-5, [128, 128], "rH")
                rH = rstdH.rearrange("p (a b) -> p a b", a=8)
                amO = po.tile([128, 8], F32, tag="amO", bufs=1)
                ssO = po.tile([128, 8], F32, tag="ssO", bufs=1)
                for t in range(8):
                    hn = po.tile([128, D], F32, tag="hn", bufs=1)
                    rb = bass.AP(tensor=rH.tensor, offset=rH[:, t, :].offset,
                                 ap=[rH.ap[0], [1, NH], [0, HD]])
                    nc.vector.tensor_tensor(
                        out=hn.rearrange("p (h d) -> p h d", h=NH),
                        in0=hT[:, t, :].rearrange("p (h d) -> p h d", h=NH),
                        in1=rb, op=AL.mult)
                    nc.vector.tensor_tensor(out=oa[:, t, :], in0=hn,
                                            in1=gs[:, t, :], op=AL.mult)
                    nc.vector.tensor_reduce(out=amO[:, t:t + 1], in_=oa[:, t, :],
                                            axis=AX.X, op=AL.max,
                                            apply_absolute_value=True)
                    scr = po.tile([128, D], F32, tag="sqo2", bufs=1)
                    nc.scalar.activation(out=scr, in_=oa[:, t, :], func=AF.Square,
                                         accum_out=ssO[:, t:t + 1])
                quant_batch(amO, ssO, 8, D, q127O, dqOo, dqOo, iw["o"], po, "qo")
                for t in range(8):
                    round_and_transpose(oa[:, t, :], q127O[:, t:t + 1], oqT,
                                        t, 8, po, "ro")

            # ------- phase C: wo matmul + residual + LN2 + mod2 + quant ----
            x2qT = big([128, 8, D], BF, "x2qT")         # s1: C..D
            xn_all = big([128, 8, D], F32, "xn_all")    # s2: C
            mod2 = big([128, 8, D], F32, "mod2")        # s3: C
            with tc.tile_pool(name="pc", bufs=2) as pcp:
                wo_sb = pcp.tile([128, 8, D], F8, tag="wosb", bufs=1)
                nc.sync.dma_start(out=wo_sb, in_=wo3[:, :, :])
                muC = pcp.tile([128, 8], F32, tag="muC", bufs=1)
                varC = pcp.tile([128, 8], F32, tag="varC", bufs=1)
                for t in range(8):
                    st = pcp.tile([128, 2, 6], F32, tag="bst2")
                    for cki, ck in enumerate(range(0, D, 512)):
                        xr2 = pcp.tile([128, 512], F32, tag="xr2")
                        nc.sync.dma_start(
                            out=xr2,
                            in_=x_sl[128 * t:128 * (t + 1), ck:ck + 512])
                        pw = pmm()
                        for j in range(8):
                            nc.tensor.matmul(pw, oqT[:, j, 128 * t:128 * (t + 1)],
                                             wo_sb[:, j, ck:ck + 512],
                                             start=(j == 0), stop=(j == 7))
                        at = pcp.tile([128, 512], F32, tag="at")
                        nc.scalar.activation(out=at, in_=pw, func=AF.Identity,
                                             scale=dqOo[:, t:t + 1])
                        ug = pcp.tile([128, 512], F32, tag="ug")
                        nc.vector.tensor_tensor(out=ug, in0=at,
                                                in1=B_g1[:, ck:ck + 512],
                                                op=AL.mult)
                        nc.vector.tensor_tensor(out=xn_all[:, t, ck:ck + 512],
                                                in0=ug, in1=xr2, op=AL.add)
                        nc.vector.bn_stats(out=st[:, cki, :],
                                           in_=xn_all[:, t, ck:ck + 512])
                    nc.sync.dma_start(out=xnew_d[128 * t:128 * (t + 1), :],
                                      in_=xn_all[:, t, :])
                    mv = pcp.tile([128, 2], F32, tag="bmv2")
                    nc.vector.bn_aggr(out=mv, in_=st)
                    nc.vector.tensor_copy(out=muC[:, t:t + 1], in_=mv[:, 0:1])
                    nc.vector.tensor_copy(out=varC[:, t:t + 1], in_=mv[:, 1:2])
                rstdC = _rsqrt(nc, pcp, varC, 1.0, 1e-6, [128, 8], "rC")
                nmrC = pcp.tile([128, 8], F32, tag="nmrC", bufs=1)
                nc.vector.tensor_tensor(out=nmrC, in0=muC, in1=rstdC, op=AL.mult)
                nc.vector.tensor_scalar(out=nmrC, in0=nmrC, scalar1=-1.0,
                                        scalar2=None, op0=AL.mult)
                amC = pcp.tile([128, 8], F32, tag="amC", bufs=1)
                ssC = pcp.tile([128, 8], F32, tag="ssC", bufs=1)
                for t in range(8):
                    u = pcp.tile([128, D], F32, tag="u2", bufs=1)
                    nc.scalar.activation(out=u, in_=xn_all[:, t, :],
                                         func=AF.Identity,
                                         scale=rstdC[:, t:t + 1],
                                         bias=nmrC[:, t:t + 1])
                    tt2 = pcp.tile([128, D], F32, tag="tt2", bufs=1)
                    nc.vector.tensor_tensor(out=tt2, in0=u, in1=B_sc2, op=AL.mult)
                    nc.vector.tensor_tensor(out=mod2[:, t, :], in0=tt2,
                                            in1=B_sh2, op=AL.add)
                    nc.vector.tensor_reduce(out=amC[:, t:t + 1],
                                            in_=mod2[:, t, :],
                                            axis=AX.X, op=AL.max,
                                            apply_absolute_value=True)
                    scr = pcp.tile([128, D], F32, tag="sqc", bufs=1)
                    nc.scalar.activation(out=scr, in_=mod2[:, t, :],
                                         func=AF.Square,
                                         accum_out=ssC[:, t:t + 1])
                quant_batch(amC, ssC, 8, D, q127C, dqCg, dqCg, iw["gate"],
                            pcp, "qc")
                for t in range(8):
                    round_and_transpose(mod2[:, t, :], q127C[:, t:t + 1], x2qT,
                                        t, 8, pcp, "rc")
            pLate.__exit__(None, None, None)

            # ---------------- phase D+E: MLP, two half-token sweeps --------
            h2a = big([128, 2, MLP], F32, "h2a")        # s0: D
            dummy_slot = big([128, 1], F32, "dummy")    # s1 skipped (x2qT live)
            h2b = big([128, 2, MLP], F32, "h2b")        # s2: D
            h2qT = big([128, 32, 512], BF, "h2qT")      # s3: D..E

            def h2_of(ti):
                return h2a[:, ti, :] if ti < 2 else h2b[:, ti - 2, :]

            with tc.tile_pool(name="pde", bufs=2) as pde:
                dw_sb = pde.tile([128, 32, D], F8, tag="dwsb", bufs=1)
                nc.sync.dma_start(out=dw_sb, in_=dw3[:, :, :])
                for half in range(2):
                    tof = 4 * half
                    amDg = pde.tile([128, 4, 8], F32, tag="amDg", bufs=2)
                    ssDg = pde.tile([128, 4, 8], F32, tag="ssDg", bufs=2)
                    for g in range(8):
                        gw_g = pde.tile([128, 8, 1024], F8, tag="gwg", bufs=2)
                        nc.sync.dma_start(out=gw_g, in_=gw4[g])
                        for ti in range(4):
                            t = tof + ti
                            pgg = pmm()
                            for j in range(8):
                                nc.tensor.matmul(
                                    pgg, x2qT[:, j, 128 * t:128 * (t + 1)],
                                    gw_g[:, j, 0:512],
                                    start=(j == 0), stop=(j == 7))
                            pyy = pmm()
                            for j in range(8):
                                nc.tensor.matmul(
                                    pyy, x2qT[:, j, 128 * t:128 * (t + 1)],
                                    gw_g[:, j, 512:1024],
                                    start=(j == 0), stop=(j == 7))
                            sil = pde.tile([128, 512], F32, tag="sil", bufs=1)
                            nc.scalar.activation(out=sil, in_=pgg, func=AF.Silu,
                                                 scale=dqCg[:, t:t + 1])
                            h2c = h2_of(ti)[:, 512 * g:512 * (g + 1)]
                            nc.vector.scalar_tensor_tensor(
                                out=h2c, in0=pyy, scalar=dqCg[:, t:t + 1],
                                in1=sil, op0=AL.mult, op1=AL.mult)
                            nc.vector.tensor_reduce(
                                out=amDg[:, ti, g:g + 1], in_=h2c,
                                axis=AX.X, op=AL.max, apply_absolute_value=True)
                            scr = pde.tile([128, 512], F32, tag="sqd", bufs=1)
                            nc.scalar.activation(
                                out=scr, in_=h2c,
                                func=AF.Square, accum_out=ssDg[:, ti, g:g + 1])
                    amD = pde.tile([128, 4], F32, tag="amD", bufs=2)
                    ssD = pde.tile([128, 4], F32, tag="ssD", bufs=2)
                    nc.vector.tensor_reduce(out=amD, in_=amDg, axis=AX.X,
                                            op=AL.max)
                    nc.vector.tensor_reduce(out=ssD, in_=ssDg, axis=AX.X,
                                            op=AL.add)
                    q127h = pde.tile([128, 4], F32, tag="q127h", bufs=2)
                    dqh = pde.tile([128, 4], F32, tag="dqh", bufs=2)
                    quant_batch(amD, ssD, 4, MLP, q127h, dqh, None, 1.0, pde,
                                "qd")
                    nc.vector.tensor_scalar(out=dqh, in0=dqh,
                                            scalar1=float(iw["down"]),
                                            scalar2=None, op0=AL.mult)
                    for ti in range(4):
                        round_and_transpose(h2_of(ti), q127h[:, ti:ti + 1],
                                            h2qT, ti, 32, pde, "rd")
                    for ti in range(4):
                        t = tof + ti
                        for ck in range(0, D, 512):
                            pdn = pmm()
                            for j2 in range(32):
                                nc.tensor.matmul(
                                    pdn, h2qT[:, j2, 128 * ti:128 * (ti + 1)],
                                    dw_sb[:, j2, ck:ck + 512],
                                    start=(j2 == 0), stop=(j2 == 31))
                            xn3 = pde.tile([128, 512], F32, tag="xn3")
                            nc.sync.dma_start(
                                out=xn3,
                                in_=xnew_d[128 * t:128 * (t + 1), ck:ck + 512])
                            v2 = pde.tile([128, 512], F32, tag="v2d", bufs=1)
                            nc.vector.scalar_tensor_tensor(
                                out=v2, in0=pdn, scalar=dqh[:, ti:ti + 1],
                                in1=B_g2[:, ck:ck + 512],
                                op0=AL.mult, op1=AL.mult)
                            outc = pde.tile([128, 512], F32, tag="outc")
                            nc.vector.tensor_tensor(out=outc, in0=v2, in1=xn3,
                                                    op=AL.add)
                            nc.sync.dma_start(
                                out=out_sl[128 * t:128 * (t + 1), ck:ck + 512],
                                in_=outc)

    nc.finalize()
    return nc


@functools.lru_cache(maxsize=2)
def _build_cached(iw_items):
    return _build(dict(iw_items))


def kernel(x, c, adaln_w, adaln_b, wi, wf, wg, gnorm_w, wo, gate_w, down_w):
    x = np.ascontiguousarray(np.asarray(x, dtype=np.float32))
    c = np.ascontiguousarray(np.asarray(c, dtype=np.float32))
    adaln_w = np.asarray(adaln_w, dtype=np.float32)
    adaln_b = np.asarray(adaln_b, dtype=np.float32)
    gnorm_w = np.asarray(gnorm_w, dtype=np.float32)

    mi, iwi = _quant_w(np.asarray(wi, dtype=np.float32))
    mf, iwf = _quant_w(np.asarray(wf, dtype=np.float32))
    mg, iwg = _quant_w(np.asarray(wg, dtype=np.float32))
    mo, iwo = _quant_w(np.asarray(wo, dtype=np.float32))
    mgate, iwgate = _quant_w(np.asarray(gate_w, dtype=np.float32))
    mdown, iwdown = _quant_w(np.asarray(down_w, dtype=np.float32))

    iw = {"i": float(iwi), "f": float(iwf), "g": float(iwg), "o": float(iwo),
          "gate": float(iwgate), "down": float(iwdown)}
    nc = _build_cached(tuple(sorted(iw.items())))

    # device layouts (see _build for index conventions)
    def stat4(w):   # [8(m), 128(p), 8(j), 128(q)]; w[oc, c]
        return np.ascontiguousarray(
            w.reshape(8, 128, 8, 128).transpose(0, 3, 2, 1).astype(FP8))

    def mov3(w):    # [128(p), 8(j), OC]; w[oc, c]
        return np.ascontiguousarray(
            w.T.reshape(8, 128, -1).transpose(1, 0, 2).astype(FP8))

    wi4_h = stat4(mi)
    wf4_h = stat4(mf)
    wg3_h = mov3(mg)
    wo3_h = mov3(mo)
    gA = mgate[:MLP].reshape(8, 512, 8, 128)     # [g, mc, j, p]
    gB = mgate[MLP:].reshape(8, 512, 8, 128)
    gw4_h = np.ascontiguousarray(np.concatenate(
        [gA.transpose(0, 3, 2, 1), gB.transpose(0, 3, 2, 1)],
        axis=3).astype(FP8))                      # [8, 128p, 8j, 1024]
    dw3_h = np.ascontiguousarray(
        mdown.T.reshape(32, 128, D).transpose(1, 0, 2).astype(FP8))

    adwT = adaln_w.T                              # [D(c), 6D(o)]
    adw_hi_f = adwT.astype(BF16).astype(np.float32)
    adw_lo_f = adwT - adw_hi_f

    def adw4(wf32):   # [12(ch), 128(p), 8(j), 512]
        return np.ascontiguousarray(
            wf32.reshape(8, 128, 12, 512).transpose(2, 1, 0, 3).astype(BF16))

    adw_hi_h = adw4(adw_hi_f)
    adw_lo_h = adw4(adw_lo_f)
    adb_row_h = np.ascontiguousarray(adaln_b.reshape(12, 1, 512))
    gnr_h = np.ascontiguousarray(np.tile(gnorm_w, NH)[None, :])

    in_maps = []
    for core in range(N_CORES):
        b, half = core // 2, core % 2
        mask = np.zeros((N_CORES, 1), np.float32)
        if half == 1:
            mask[core - 1, 0] = 1.0
        c_col_h = np.ascontiguousarray(c[b].reshape(8, 128).T)   # [128(p), 8(j)]
        in_maps.append({
            "x_sl": np.ascontiguousarray(x[b, half * TOK:(half + 1) * TOK, :]),
            "c_col": c_col_h,
            "adw_hi": adw_hi_h,
            "adw_lo": adw_lo_h,
            "adb_row": adb_row_h,
            "mask8": mask,
            "gnr": gnr_h,
            "wi4": wi4_h, "wf4": wf4_h, "wg3": wg3_h, "wo3": wo3_h,
            "gw4": gw4_h, "dw3": dw3_h,
        })

    res = run_bass_kernel_spmd(nc, in_maps, core_ids=list(range(N_CORES)))
    out = np.zeros((B, T, D), np.float32)
    for core in range(N_CORES):
        b, half = core // 2, core % 2
        out[b, half * TOK:(half + 1) * TOK, :] = res.results[core]["out_sl"]
    return out
